# revision 7
# baseline (speedup 1.0000x reference)
"""LocalIsing energy kernel for Trainium2 (8 NeuronCores, data-parallel over batch).

reference:  energy[b] = x[b] @ J1 + sum_c J2[c] * x[b, p0[c]] * x[b, p1[c]]

The pair term is a quadratic form: scatter-add J2 into an upper-triangular
W[512,512] at (min(p0,p1), max(p0,p1)) host-side, then
    energy[b] = sum_j x[b,j] * ((x @ W)[b,j] + J1[j])
J1 rides along as a K=1 matmul tile (ones row x J1 row) accumulated into the
same PSUM bank, so the whole energy is one fused DVE multiply+reduce.

W is strictly block-upper-triangular over 128x128 tiles, so only the 10
nonzero tiles ship and each K-tile matmul shrinks its moving dimension:
row-tile k covers columns [128k, 512) (N = 512-128k). The J1 matmul runs
first with start=True over the full width to zero the PSUM bank.

All operands travel as bf16 (x is exactly representable; W/J1 rounding gives
~0.3% relative error, far under the 2e-2 gate). Per core one packed DRAM blob
[128, 2432] bf16 keeps every partition line contiguous (4864B descriptors):
  per partition p: W row chunks (512+384+256+128) | x^T cols (4 x 128)
                   | x row (512) | identity row (128, output transpose)
The [128,1] energy column is PE-transposed to [1,128] so the result leaves in
a single 256B DMA packet instead of 128 4-byte packets.

Scheduling details:
- The blob DMA launches on the SP hwdge queue; the small J1/ones DMA goes
  through the gpsimd SWDGE queue so the two launches don't serialize and the
  transfers don't share the hw DGE FIFO (a concurrent hwdge pair was measured
  to starve the small transfer until the big one finished).
- The PE runs dummy matmuls on a memset scratch tile while the blob DMA is in
  flight: TRN2's PE clock ramps with sustained busy time, so warming it
  shortens the real matmuls on the critical path.
"""

import numpy as np
from contextlib import ExitStack

import ml_dtypes
import concourse.tile as tile
from concourse import bacc, mybir
from concourse.bass_utils import run_bass_kernel_spmd

N = 512          # spins
B = 1024         # batch
NCORES = 8
BS = B // NCORES  # 128 rows per core = one partition tile
KT = N // 128     # 4 contraction tiles

BF16 = ml_dtypes.bfloat16

# blob column offsets (bf16 elements); W row-tile k spans columns [128k, 512)
_RT_W = [N - 128 * k for k in range(KT)]          # 512, 384, 256, 128
_W_OFF = [0, 512, 896, 1152]                      # cumsum of _RT_W
_XT_OFF = 1280             # 4 tiles x 128
_X_OFF = _XT_OFF + N       # 1792: x row (512)
_ID_OFF = _X_OFF + N       # 2304: identity row (128)
_BLOB_W = _ID_OFF + 128    # 2432

WARM_BIG = 10              # N=512 dummies: carry the PE through the ramp
WARM_SMALL = 4             # N=64 dummies: fine-grained busy filler

_cached_nc = None


def _build():
    bf16 = mybir.dt.bfloat16
    f32 = mybir.dt.float32
    nc = bacc.Bacc(
        "TRN2", target_bir_lowering=False, debug=False, num_devices=1
    )
    blob = nc.dram_tensor("blob", [128, _BLOB_W], bf16, kind="ExternalInput")
    cst = nc.dram_tensor("cst", [1, N + 128], bf16, kind="ExternalInput")
    en = nc.dram_tensor("energy", [1, BS], bf16, kind="ExternalOutput")

    with tile.TileContext(nc) as tc, ExitStack() as ctx:
        sb = ctx.enter_context(tc.tile_pool(name="sb", bufs=1))
        ps = ctx.enter_context(tc.tile_pool(name="ps", bufs=1, space="PSUM"))

        # warmup scratch memset goes first so the PE dummies can start early
        wsrc = sb.tile([128, 640], bf16)
        nc.gpsimd.memset(wsrc, 0)

        # cst launches right after blob on the same engine: all DMA paths
        # drain through one per-core DGE FIFO, so the tiny cst transfer lands
        # immediately behind the blob either way; launching it elsewhere only
        # adds overhead.
        blob_sb = sb.tile([128, _BLOB_W], bf16)
        nc.sync.dma_start(blob_sb, blob[:, :])
        cst_sb = sb.tile([1, N + 128], bf16)
        nc.sync.dma_start(cst_sb, cst[:, :], single_packet=True)
        wps = ps.tile([128, N], f32)
        for _ in range(WARM_BIG):
            nc.tensor.matmul(wps, wsrc[:, :128], wsrc[:, 128:640], start=True, stop=True)
        for _ in range(WARM_SMALL):
            nc.tensor.matmul(wps[:, :64], wsrc[:, :128], wsrc[:, 128:192], start=True, stop=True)

        # y = 1 (x) J1  +  x @ W; the K=1 J1 tile goes first (start=True over
        # the full width zeroes the bank) and only needs cst, so it runs while
        # the blob DMA is still in flight.
        y = ps.tile([128, N], f32)
        nc.tensor.matmul(
            y, cst_sb[:1, N : N + 128], cst_sb[:1, :N], start=True, stop=False
        )
        for k in range(KT):
            nc.tensor.matmul(
                y[:, 128 * k : N],
                blob_sb[:, _XT_OFF + k * 128 : _XT_OFF + (k + 1) * 128],
                blob_sb[:, _W_OFF[k] : _W_OFF[k] + _RT_W[k]],
                start=False,
                stop=(k == KT - 1),
            )

        # e[b] = sum_j y[b,j] * x[b,j]  (single fused DVE mul+reduce;
        # tensor_tensor_reduce miscompiles on HW, scalar_tensor_tensor's
        # accum_out path does not)
        scr = sb.tile([128, N], f32)
        e32 = sb.tile([128, 1], f32)
        nc.vector.scalar_tensor_tensor(
            out=scr,
            in0=y,
            scalar=1.0,
            in1=blob_sb[:, _X_OFF : _X_OFF + N],
            op0=mybir.AluOpType.mult,
            op1=mybir.AluOpType.mult,
            accum_out=e32,
        )

        # [128,1] -> [1,128] via PE transpose so the output leaves as one packet
        e16 = sb.tile([128, 1], bf16)
        nc.vector.tensor_copy(e16, e32)
        et = ps.tile([1, 128], bf16)
        nc.tensor.transpose(et, e16, blob_sb[:, _ID_OFF : _ID_OFF + 128])
        erow = sb.tile([1, 128], bf16)
        nc.vector.tensor_copy(erow, et)
        nc.scalar.dma_start(en[:, :], erow, single_packet=True)
    nc.finalize()
    return nc


def _pack_inputs(x, J1, J2, pairs):
    x = np.asarray(x, dtype=np.float32)
    J1 = np.asarray(J1, dtype=np.float32)
    J2f = np.asarray(J2, dtype=np.float64)
    pairs = np.asarray(pairs)

    # Scatter-add J2 into upper-triangular W (min,max fold handles pairs in
    # either order; duplicates and diagonal pairs accumulate exactly like the
    # reference's gather-sum).
    lo = np.minimum(pairs[:, 0], pairs[:, 1]).astype(np.int64)
    hi = np.maximum(pairs[:, 0], pairs[:, 1]).astype(np.int64)
    W = np.bincount(lo * N + hi, weights=J2f, minlength=N * N).astype(np.float32)
    Wb = W.reshape(N, N).astype(BF16)
    # row-tile k keeps only columns [128k, 512)
    Wrows = np.concatenate(
        [Wb[128 * k : 128 * (k + 1), 128 * k :] for k in range(KT)], axis=1
    )  # [128, 1280]
    eye = np.eye(128, dtype=BF16)
    cst = np.concatenate([J1.astype(BF16), np.ones(128, dtype=BF16)])[None, :]

    in_maps = []
    for c in range(NCORES):
        shard = x[c * BS : (c + 1) * BS].astype(BF16)
        blob = np.empty((128, _BLOB_W), dtype=BF16)
        blob[:, :_XT_OFF] = Wrows
        # lhsT tile k, partition p holds x_shard[:, 128k+p]
        blob[:, _XT_OFF:_X_OFF] = np.ascontiguousarray(
            shard.T.reshape(KT, 128, BS).transpose(1, 0, 2).reshape(128, KT * BS)
        )
        blob[:, _X_OFF:_ID_OFF] = shard
        blob[:, _ID_OFF:] = eye
        in_maps.append({"blob": blob, "cst": cst})
    return in_maps


def kernel(x, J1, J2, pairs):
    global _cached_nc
    if _cached_nc is None:
        _cached_nc = _build()
    in_maps = _pack_inputs(x, J1, J2, pairs)
    res = run_bass_kernel_spmd(_cached_nc, in_maps, core_ids=list(range(NCORES)))
    return np.concatenate(
        [r["energy"].reshape(-1).astype(np.float32) for r in res.results]
    )


# revision 10
# speedup vs baseline: 1.0837x; 1.0837x over previous
"""LocalIsing energy kernel for Trainium2 (8 NeuronCores, data-parallel over batch).

reference:  energy[b] = x[b] @ J1 + sum_c J2[c] * x[b, p0[c]] * x[b, p1[c]]

The pair term is a quadratic form: scatter-add J2 into an upper-triangular
W[512,512] at (min(p0,p1), max(p0,p1)) host-side, then
    energy[b] = sum_j x[b,j] * ((x @ W)[b,j] + J1[j])
J1 rides along as a K=1 matmul tile (ones row x J1 row) accumulated into the
same PSUM bank, so the whole energy is one fused DVE multiply+reduce.

W is strictly block-upper-triangular over 128x128 tiles, so only the 10
nonzero tiles ship and each K-tile matmul shrinks its moving dimension:
row-tile k covers columns [128k, 512) (N = 512-128k). The J1 matmul runs
first with start=True over the full width to zero the PSUM bank.

All matmul operands travel as bf16 (x is exactly representable; W/J1 rounding
gives ~0.3% relative error, far under the 2e-2 gate). Per core one packed
DRAM blob [128, 2304] bf16 keeps every partition line contiguous (4608B
descriptors):
  per partition p: W row chunks (512+384+256+128) | x^T cols (4 x 128)
                   | x row (512)
The [128,1] f32 energy column is block-transposed on the DVE (StreamTranspose,
32x32 blocks) so the result leaves as four 128B descriptors from partitions
{0,32,64,96} instead of 128 4-byte packets; the host flattens [4,32] -> [128].

Scheduling details:
- All DMA paths drain through one per-core DGE FIFO, so the tiny J1/ones cst
  transfer launches right after the blob on the same engine and lands just
  behind it; J1's matmul pipelines with the first W matmul's weight load.
- The PE runs dummy matmuls on a memset scratch tile while the blob DMA is in
  flight: TRN2's PE clock ramps with sustained busy time, so warming it
  shortens the real matmuls on the critical path.
"""

import numpy as np
from contextlib import ExitStack

import ml_dtypes
import concourse.tile as tile
from concourse import bacc, mybir
from concourse.bass_utils import run_bass_kernel_spmd

N = 512          # spins
B = 1024         # batch
NCORES = 8
BS = B // NCORES  # 128 rows per core = one partition tile
KT = N // 128     # 4 contraction tiles

BF16 = ml_dtypes.bfloat16

# blob column offsets (bf16 elements); W row-tile k spans columns [128k, 512)
_RT_W = [N - 128 * k for k in range(KT)]          # 512, 384, 256, 128
_W_OFF = [0, 512, 896, 1152]                      # cumsum of _RT_W
_XT_OFF = 1280             # 4 tiles x 128
_X_OFF = _XT_OFF + N       # 1792: x row (512)
_BLOB_W = _X_OFF + N       # 2304

WARM_BIG = 8               # N=512 dummies: carry the PE through the ramp
WARM_SMALL = 3             # N=64 dummies: fine-grained busy filler

_cached_nc = None


def _build():
    bf16 = mybir.dt.bfloat16
    f32 = mybir.dt.float32
    nc = bacc.Bacc(
        "TRN2", target_bir_lowering=False, debug=False, num_devices=1
    )
    blob = nc.dram_tensor("blob", [128, _BLOB_W], bf16, kind="ExternalInput")
    cst = nc.dram_tensor("cst", [1, N + 128], bf16, kind="ExternalInput")
    en = nc.dram_tensor("energy", [4, 32], f32, kind="ExternalOutput")

    with tile.TileContext(nc) as tc, ExitStack() as ctx:
        sb = ctx.enter_context(tc.tile_pool(name="sb", bufs=1))
        ps = ctx.enter_context(tc.tile_pool(name="ps", bufs=1, space="PSUM"))

        # warmup scratch memset goes first so the PE dummies can start early
        wsrc = sb.tile([128, 640], bf16)
        nc.gpsimd.memset(wsrc, 0)
        # energy staging tile: only column 0 is written by the reduce, but the
        # StreamTranspose reads (and the simulator checks) all 32 columns
        ecol = sb.tile([128, 32], f32)
        nc.gpsimd.memset(ecol, 0)

        # cst launches right after blob on the same engine: all DMA paths
        # drain through one per-core DGE FIFO, so the tiny cst transfer lands
        # immediately behind the blob either way; launching it elsewhere only
        # adds overhead.
        blob_sb = sb.tile([128, _BLOB_W], bf16)
        nc.sync.dma_start(blob_sb, blob[:, :])
        cst_sb = sb.tile([1, N + 128], bf16)
        nc.sync.dma_start(cst_sb, cst[:, :], single_packet=True)

        # PE p-state warmup while the blob DMA is in flight
        wps = ps.tile([128, N], f32)
        for _ in range(WARM_BIG):
            nc.tensor.matmul(wps, wsrc[:, :128], wsrc[:, 128:640], start=True, stop=True)
        for _ in range(WARM_SMALL):
            nc.tensor.matmul(wps[:, :64], wsrc[:, :128], wsrc[:, 128:192], start=True, stop=True)

        # y = 1 (x) J1  +  x @ W; the K=1 J1 tile goes first (start=True over
        # the full width zeroes the bank) and pipelines with mm0's LDWEIGHTS.
        y = ps.tile([128, N], f32)
        nc.tensor.matmul(
            y, cst_sb[:1, N : N + 128], cst_sb[:1, :N], start=True, stop=False
        )
        for k in range(KT):
            nc.tensor.matmul(
                y[:, 128 * k : N],
                blob_sb[:, _XT_OFF + k * 128 : _XT_OFF + (k + 1) * 128],
                blob_sb[:, _W_OFF[k] : _W_OFF[k] + _RT_W[k]],
                start=False,
                stop=(k == KT - 1),
            )

        # e[b] = sum_j y[b,j] * x[b,j]  (single fused DVE mul+reduce;
        # tensor_tensor_reduce miscompiles on HW, scalar_tensor_tensor's
        # accum_out path does not). accum lands in column 0 of a [128,32]
        # tile that the DVE then block-transposes: energies for batch rows
        # 32r..32r+31 end up in partition 32r, columns 0..31.
        scr = sb.tile([128, N], f32)
        nc.vector.scalar_tensor_tensor(
            out=scr,
            in0=y,
            scalar=1.0,
            in1=blob_sb[:, _X_OFF : _X_OFF + N],
            op0=mybir.AluOpType.mult,
            op1=mybir.AluOpType.mult,
            accum_out=ecol[:, 0:1],
        )
        erow = sb.tile([128, 32], f32)
        nc.vector.transpose(erow, ecol)
        nc.scalar.dma_start(en[:, :], erow[0:128:32, 0:32], single_packet=True)
    nc.finalize()
    return nc


def _pack_inputs(x, J1, J2, pairs):
    x = np.asarray(x, dtype=np.float32)
    J1 = np.asarray(J1, dtype=np.float32)
    J2f = np.asarray(J2, dtype=np.float64)
    pairs = np.asarray(pairs)

    # Scatter-add J2 into upper-triangular W (min,max fold handles pairs in
    # either order; duplicates and diagonal pairs accumulate exactly like the
    # reference's gather-sum).
    lo = np.minimum(pairs[:, 0], pairs[:, 1]).astype(np.int64)
    hi = np.maximum(pairs[:, 0], pairs[:, 1]).astype(np.int64)
    W = np.bincount(lo * N + hi, weights=J2f, minlength=N * N).astype(np.float32)
    Wb = W.reshape(N, N).astype(BF16)
    # row-tile k keeps only columns [128k, 512)
    Wrows = np.concatenate(
        [Wb[128 * k : 128 * (k + 1), 128 * k :] for k in range(KT)], axis=1
    )  # [128, 1280]
    cst = np.concatenate([J1.astype(BF16), np.ones(128, dtype=BF16)])[None, :]

    in_maps = []
    for c in range(NCORES):
        shard = x[c * BS : (c + 1) * BS].astype(BF16)
        blob = np.empty((128, _BLOB_W), dtype=BF16)
        blob[:, :_XT_OFF] = Wrows
        # lhsT tile k, partition p holds x_shard[:, 128k+p]
        blob[:, _XT_OFF:_X_OFF] = np.ascontiguousarray(
            shard.T.reshape(KT, 128, BS).transpose(1, 0, 2).reshape(128, KT * BS)
        )
        blob[:, _X_OFF:] = shard
        in_maps.append({"blob": blob, "cst": cst})
    return in_maps


def kernel(x, J1, J2, pairs):
    global _cached_nc
    if _cached_nc is None:
        _cached_nc = _build()
    in_maps = _pack_inputs(x, J1, J2, pairs)
    res = run_bass_kernel_spmd(_cached_nc, in_maps, core_ids=list(range(NCORES)))
    return np.concatenate(
        [r["energy"].reshape(-1).astype(np.float32) for r in res.results]
    )
